# revision 9
# baseline (speedup 1.0000x reference)
"""Trainium2 Bass kernel for MllamaTextSelfAttention (B=1, S=2048, HID=4096,
32 Q heads / 8 KV heads, HD=128, RoPE, causal mask, GQA).

Sharding: tensor-parallel over heads across 8 NeuronCores. Core c computes
Q heads [4c, 4c+4) and KV head c, plus the matching slice of the output
projection; the 8 partial outputs are summed on the host.

All DRAM I/O is bf16 (host converts); matmuls run bf16 at 1 cyc/row.

Device dataflow (per core):
  phase 1 (per 512-seq stripe, pipelined):
    - stream ht k-tiles, interleaved with the wq/wk/wv weight tiles on the
      first stripe (3 DMA queues round-robin) so PE starts within ~1us
    - qT/kT/vT stationary-weight projections -> [d, s] layouts in PSUM
    - PSUM->SBUF copies on Act (bf16), RoPE per stripe on DVE,
      V transposed per stripe via PE into v2 [s, t*(d|1)] with a ones column
    - wo prefetched during stripe 1
  phase 2+3 (per q stripe qs, heads software-pipelined):
    - S^T[k, q] = kT-tile.T @ qT-stripe (k-partition layout); diagonal tiles
      computed only for the causal column range
    - E = exp(S^T + tri-mask) on Act -> bf16 e tiles
    - O[q, d|den] = e-subtile.T @ (v|1)-tile accumulated over k: the ones
      column of v2 makes column 128 the softmax denominator (no separate
      den matmul)
    - normalize on DVE (per-partition reciprocal bcast), transpose O via PE
      back to [d, q] for the output projection
    - phase 3 for this stripe's 4 s-tiles: Y stripe = O^T-tiles.T @ woT,
      streamed to DRAM as bf16
"""

import math
import os
import sys

for _p in (
    "/opt/trn_rl_repo",
    "/root/.axon_site",
    "/root/.axon_site/_ro/trn_rl_repo",
    "/root/.axon_site/_ro/pypackages",
):
    if os.path.isdir(_p) and _p not in sys.path:
        sys.path.append(_p)

import numpy as np
import ml_dtypes
from contextlib import ExitStack

import concourse.bass as bass
import concourse.tile as tile
from concourse import mybir
from concourse.bass_utils import run_bass_kernel_spmd
from concourse.masks import make_identity

F32 = mybir.dt.float32
BF16 = mybir.dt.bfloat16
ACTF = mybir.ActivationFunctionType
MULT = mybir.AluOpType.mult
BF = ml_dtypes.bfloat16

B, S, HID = 1, 2048, 4096
NH, NKV, HD = 32, 8, 128
NCORES = 8
QH = NH // NCORES          # 4 q heads per core
SS = 512                   # sequence stripe (matmul free dim)
NQS = S // SS              # 4 stripes
NKT = S // 128             # 16 k tiles
KH = HID // 128            # 32 hidden-dim k tiles
VW = 130                   # v2 column stride per k-tile (128 d + ones + pad)
NEG = -1e9


def _split_multi_waits(nc: bass.Bass):
    """Walrus in this container encodes at most ONE sync-wait command per
    instruction. Hoist extra waits onto injected same-engine NoOps placed
    immediately before the instruction; engines are in-order so the
    semantics are unchanged."""
    n = 0
    for fn in nc.m.functions:
        for bb in fn.blocks:
            out = []
            for inst in bb.instructions:
                si = inst.sync_info
                if si is not None and si.on_wait and len(si.on_wait) > 1:
                    waits = list(si.on_wait)
                    for w in waits[:-1]:
                        n += 1
                        nop = mybir.InstNoOp(name=f"I-swait-{n}", ins=[], outs=[])
                        nop.engine = inst.engine
                        nop.sync_info = mybir.SyncInfo(on_wait=[w], on_update=[])
                        out.append(nop)
                    si.on_wait = [waits[-1]]
                out.append(inst)
            bb.instructions[:] = out
    return nc


_BUILD_CACHE = {}


def _build(causal: bool, split_waits: bool = True, loop_n=None) -> bass.Bass:
    key = (causal, split_waits, loop_n)
    if key in _BUILD_CACHE:
        return _BUILD_CACHE[key]

    nc = bass.Bass()
    hT = nc.dram_tensor("hT", [HID, S], BF16, kind="ExternalInput")
    wqT = nc.dram_tensor("wqT", [HID, QH * HD], BF16, kind="ExternalInput")
    wkT = nc.dram_tensor("wkT", [HID, HD], BF16, kind="ExternalInput")
    wvT = nc.dram_tensor("wvT", [HID, HD], BF16, kind="ExternalInput")
    woT = nc.dram_tensor("woT", [QH * HD, HID], BF16, kind="ExternalInput")
    cosT = nc.dram_tensor("cosT", [HD, S], BF16, kind="ExternalInput")
    sinT = nc.dram_tensor("sinT", [HD, S], BF16, kind="ExternalInput")
    tri = nc.dram_tensor("tri", [128, 128], F32, kind="ExternalInput")
    if not causal:
        maskT = nc.dram_tensor("maskT", [S, S], F32, kind="ExternalInput")
    y = nc.dram_tensor("y", [S, HID], BF16, kind="ExternalOutput")

    with tile.TileContext(nc) as tc, ExitStack() as ctx:
        if loop_n is not None:
            ctx.enter_context(tc.For_i(0, loop_n, 1))
        # the two HWDGE-capable issue queues, round-robined for bulk DMA
        # (gpsimd/SWDGE is avoided: it can't run inside For_i)
        DQ = [nc.sync, nc.scalar]
        qi = [0]

        def dq():
            e = DQ[qi[0] % len(DQ)]
            qi[0] += 1
            return e

        outer = ctx.enter_context(tc.tile_pool(name="outer", bufs=1))
        ppt = ctx.enter_context(tc.tile_pool(name="ppt", bufs=2, space="PSUM"))

        qT = outer.tile([128, QH * S], BF16)     # [d, h*s]
        kT = outer.tile([128, S], BF16)          # [d, s]
        v2 = outer.tile([128, NKT * VW], BF16)   # [s-in-tile, t*(d|1|pad)]
        ot = outer.tile([128, QH * S], BF16)     # [d, h*s] normalized O^T
        cos_sb = outer.tile([128, S], BF16)
        sin_sb = outer.tile([128, S], BF16)
        wo_sb = outer.tile([128, QH * HID], BF16)
        id_bf = outer.tile([128, 128], BF16)
        tri_sb = outer.tile([128, 128], F32)

        make_identity(nc, id_bf[:])
        for t in range(NKT):
            nc.gpsimd.memset(v2[:, t * VW + 128 : t * VW + 129], 1.0)

        # ---------------- phase 1: QKV projections (pipelined) -------------
        with (
            tc.tile_pool(name="wqkv", bufs=1) as wp,
            tc.tile_pool(name="hstream", bufs=6) as hp,
            tc.tile_pool(name="vtmp", bufs=2) as vp,
            tc.tile_pool(name="rtmp", bufs=2) as rp,
            tc.tile_pool(name="ps1", bufs=1, space="PSUM") as pp1,
        ):
            wq_c = [wp.tile([128, 512], BF16, name=f"wqc{k}", tag=f"wqc{k}") for k in range(KH)]
            wk_c = [wp.tile([128, HD], BF16, name=f"wkc{k}", tag=f"wkc{k}") for k in range(KH)]
            wv_c = [wp.tile([128, HD], BF16, name=f"wvc{k}", tag=f"wvc{k}") for k in range(KH)]

            def rope(dst, n):
                """RoPE in [d, s] layout on a [128, SS] bf16 stripe, in place.
                rotate_half: rot[0:64] = -src[64:128]; rot[64:128] = src[0:64].
                dst = dst*cos + rot(dst)*sin"""
                c = cos_sb[:, n * SS : (n + 1) * SS]
                sn = sin_sb[:, n * SS : (n + 1) * SS]
                t1 = rp.tile([128, SS], BF16, tag="t1")
                t2 = rp.tile([128, SS], BF16, tag="t2")
                nc.vector.tensor_scalar_mul(t1[0:64, :], dst[64:128, :], -1.0)
                nc.vector.tensor_copy(t1[64:128, :], dst[0:64, :])
                nc.vector.tensor_mul(t1[:], t1[:], sn)
                nc.vector.tensor_mul(t2[:], dst, c)
                nc.vector.tensor_add(dst, t1[:], t2[:])

            vts = []  # pending (vtile, n) for PE transposes

            def v_transpose(vt, n):
                for j in range(4):
                    t = 4 * n + j
                    pt = ppt.tile([128, 128], BF16, tag="pt")
                    nc.tensor.transpose(pt[:], vt[:, j * 128 : (j + 1) * 128], id_bf[:])
                    nc.vector.tensor_copy(v2[:, t * VW : t * VW + 128], pt[:])

            for n in range(NQS):
                psq = [
                    pp1.tile([128, SS], F32, name=f"psq{m}", tag=f"psq{m}")
                    for m in range(QH)
                ]
                psk = pp1.tile([128, SS], F32, tag="psk")
                psv = pp1.tile([128, SS], F32, tag="psv")
                for k in range(KH):
                    ht = hp.tile([128, SS], BF16, tag="ht")
                    if n == 0:
                        dq().dma_start(wq_c[k][:], wqT[k * 128 : (k + 1) * 128, :])
                        dq().dma_start(wk_c[k][:], wkT[k * 128 : (k + 1) * 128, :])
                        dq().dma_start(wv_c[k][:], wvT[k * 128 : (k + 1) * 128, :])
                    dq().dma_start(
                        ht[:], hT[k * 128 : (k + 1) * 128, n * SS : (n + 1) * SS]
                    )
                    if n == 0 and k == 8:
                        # RoPE tables + mask aren't needed until the first
                        # stripe completes; keep them off the critical path
                        dq().dma_start(cos_sb[:], cosT[:, :])
                        dq().dma_start(sin_sb[:], sinT[:, :])
                        dq().dma_start(tri_sb[:], tri[:, :])
                    st, sp = (k == 0), (k == KH - 1)
                    for m in range(QH):
                        nc.tensor.matmul(
                            psq[m][:],
                            wq_c[k][:, m * 128 : (m + 1) * 128],
                            ht[:],
                            start=st,
                            stop=sp,
                        )
                    nc.tensor.matmul(psk[:], wk_c[k][:], ht[:], start=st, stop=sp)
                    nc.tensor.matmul(psv[:], wv_c[k][:], ht[:], start=st, stop=sp)
                    # prefetch wo during stripe 1 (queues are idle by then)
                    if n == 1 and k < QH:
                        dq().dma_start(
                            wo_sb[:, k * HID : (k + 1) * HID],
                            woT[k * 128 : (k + 1) * 128, :],
                        )
                # PSUM -> SBUF (bf16) on Act, then RoPE stripe on DVE
                for m in range(QH):
                    nc.scalar.copy(
                        qT[:, m * S + n * SS : m * S + (n + 1) * SS], psq[m][:]
                    )
                nc.scalar.copy(kT[:, n * SS : (n + 1) * SS], psk[:])
                vt = vp.tile([128, SS], BF16, tag="vt")
                nc.scalar.copy(vt[:], psv[:])
                for m in range(QH):
                    rope(qT[:, m * S + n * SS : m * S + (n + 1) * SS], n)
                rope(kT[:, n * SS : (n + 1) * SS], n)
                # delay transposes one stripe so the Act copy has drained
                vts.append((vt, n))
                if len(vts) > 1:
                    v_transpose(*vts.pop(0))
            while vts:
                v_transpose(*vts.pop(0))

        # ---------------- phase 2+3: attention + output projection ---------
        with (
            tc.tile_pool(name="epool", bufs=2) as ep,
            tc.tile_pool(name="onat", bufs=2) as op_,
            tc.tile_pool(name="recp", bufs=4) as rcp,
            tc.tile_pool(name="mrowp", bufs=1) as mp,
            tc.tile_pool(name="yout", bufs=2) as yp,
            tc.tile_pool(name="ps2s", bufs=2, space="PSUM") as pp2s,
            tc.tile_pool(name="ps2o", bufs=2, space="PSUM") as pp2o,
            tc.tile_pool(name="ps3", bufs=2, space="PSUM") as pp3,
        ):
            for qs in range(NQS):
                nkt = 4 * qs + 4 if causal else NKT
                if not causal:
                    mrow = mp.tile([128, NKT * SS], F32, tag="mrow")
                    for t in range(NKT):
                        dq().dma_start(
                            mrow[:, t * SS : (t + 1) * SS],
                            maskT[t * 128 : (t + 1) * 128, qs * SS : (qs + 1) * SS],
                        )

                etiles = {}

                def emit_scores(h):
                    e = ep.tile([128, NKT * SS], BF16, tag="e")
                    etiles[h] = e
                    qsl = qT[:, h * S + qs * SS : h * S + (qs + 1) * SS]
                    for t in range(nkt):
                        pss = pp2s.tile([128, SS], F32, tag="pss")
                        diag_j = t - 4 * qs if causal and t >= 4 * qs else None
                        if diag_j is not None and diag_j > 0:
                            # columns < 128*diag_j are strictly above the
                            # causal diagonal and never read downstream
                            c0 = 128 * diag_j
                        else:
                            c0 = 0
                        nc.tensor.matmul(
                            pss[:, c0:SS],
                            kT[:, t * 128 : (t + 1) * 128],
                            qsl[:, c0:SS],
                            start=True,
                            stop=True,
                        )
                        dst = e[:, t * SS + c0 : (t + 1) * SS]
                        if diag_j is not None:
                            nc.vector.tensor_add(
                                pss[:, c0 : c0 + 128],
                                pss[:, c0 : c0 + 128],
                                tri_sb[:],
                            )
                            nc.scalar.activation(dst, pss[:, c0:SS], ACTF.Exp)
                        elif not causal:
                            nc.vector.tensor_add(
                                pss[:], pss[:], mrow[:, t * SS : (t + 1) * SS]
                            )
                            nc.scalar.activation(dst, pss[:], ACTF.Exp)
                        else:
                            nc.scalar.activation(dst, pss[:], ACTF.Exp)

                def emit_pv(h):
                    e = etiles.pop(h)
                    for j in range(QH):
                        qt = 4 * qs + j          # global q 128-tile index
                        nt = qt + 1 if causal else NKT
                        po = pp2o.tile([128, 132], F32, tag="po")
                        for t in range(nt):
                            nc.tensor.matmul(
                                po[:, 0:129],
                                e[:, t * SS + j * 128 : t * SS + (j + 1) * 128],
                                v2[:, t * VW : t * VW + 129],
                                start=(t == 0),
                                stop=(t == nt - 1),
                            )
                        rec = rcp.tile([128, 1], F32, tag="rec")
                        nc.vector.reciprocal(rec[:], po[:, 128:129])
                        on = op_.tile([128, 128], BF16, tag="on")
                        nc.vector.tensor_scalar_mul(on[:], po[:, 0:128], rec[:, 0:1])
                        pt = ppt.tile([128, 128], BF16, tag="pt")
                        nc.tensor.transpose(pt[:], on[:], id_bf[:])
                        nc.vector.tensor_copy(
                            ot[:, h * S + qt * 128 : h * S + (qt + 1) * 128], pt[:]
                        )

                # software-pipeline heads: scores(h+1) overlaps exp(h) drain
                emit_scores(0)
                for h in range(1, QH):
                    emit_scores(h)
                    emit_pv(h - 1)
                emit_pv(QH - 1)

                # phase 3 for this stripe's 4 s-tiles
                for j in range(4):
                    st = 4 * qs + j
                    yt = yp.tile([128, HID], BF16, tag="yt")
                    for nn in range(HID // SS):
                        psy = pp3.tile([128, SS], F32, tag="psy")
                        for hh in range(QH):
                            nc.tensor.matmul(
                                psy[:],
                                ot[:, hh * S + st * 128 : hh * S + (st + 1) * 128],
                                wo_sb[:, hh * HID + nn * SS : hh * HID + (nn + 1) * SS],
                                start=(hh == 0),
                                stop=(hh == QH - 1),
                            )
                        if nn % 2 == 0:
                            nc.scalar.copy(yt[:, nn * SS : (nn + 1) * SS], psy[:])
                        else:
                            nc.vector.tensor_copy(yt[:, nn * SS : (nn + 1) * SS], psy[:])
                    nc.sync.dma_start(y[st * 128 : (st + 1) * 128, :], yt[:])

    if split_waits:
        _split_multi_waits(nc)
    _BUILD_CACHE[key] = nc
    return nc


def _causal_mask_ref() -> np.ndarray:
    return np.triu(np.full((S, S), NEG, np.float32), k=1)


def _tri_mask() -> np.ndarray:
    p = np.arange(128, dtype=np.int64)[:, None]
    f = np.arange(128, dtype=np.int64)[None, :]
    return np.where(p > f, np.float32(NEG), np.float32(0.0)).astype(np.float32)


def make_in_maps(hidden_states, attention_mask, cos, sin, wq, wk, wv, wo):
    """Host-side sharding/preprocessing. Returns (causal, in_maps)."""
    h = np.ascontiguousarray(np.asarray(hidden_states, dtype=np.float32)[0])
    m2 = np.ascontiguousarray(np.asarray(attention_mask, dtype=np.float32)[0, 0])
    wq = np.asarray(wq, dtype=np.float32)
    wk = np.asarray(wk, dtype=np.float32)
    wv = np.asarray(wv, dtype=np.float32)
    wo = np.asarray(wo, dtype=np.float32)

    causal = bool(np.array_equal(m2, _causal_mask_ref()))
    hT = np.ascontiguousarray(h.T.astype(BF))
    cosT = np.ascontiguousarray(np.asarray(cos, dtype=np.float32)[0].T.astype(BF))
    sinT = np.ascontiguousarray(np.asarray(sin, dtype=np.float32)[0].T.astype(BF))
    sc = np.float32(1.0 / math.sqrt(HD))
    trim = _tri_mask()
    if not causal:
        mT = np.ascontiguousarray(m2.T)

    in_maps = []
    for c in range(NCORES):
        im = {
            "hT": hT,
            "cosT": cosT,
            "sinT": sinT,
            "tri": trim,
            "wqT": np.ascontiguousarray(
                (wq[c * QH * HD : (c + 1) * QH * HD] * sc).T.astype(BF)
            ),
            "wkT": np.ascontiguousarray(wk[c * HD : (c + 1) * HD].T.astype(BF)),
            "wvT": np.ascontiguousarray(wv[c * HD : (c + 1) * HD].T.astype(BF)),
            "woT": np.ascontiguousarray(
                wo[:, c * QH * HD : (c + 1) * QH * HD].T.astype(BF)
            ),
        }
        if not causal:
            im["maskT"] = mT
        in_maps.append(im)
    return causal, in_maps


def kernel(hidden_states, attention_mask, cos, sin, wq, wk, wv, wo):
    causal, in_maps = make_in_maps(
        hidden_states, attention_mask, cos, sin, wq, wk, wv, wo
    )
    nc = _build(causal)
    res = run_bass_kernel_spmd(nc, in_maps, list(range(NCORES)))
    out = np.zeros((S, HID), np.float64)
    for c in range(NCORES):
        out += np.asarray(res.results[c]["y"]).astype(np.float64)
    return out.reshape(B, S, HID).astype(np.float32)


# revision 23
# speedup vs baseline: 1.0201x; 1.0201x over previous
"""Trainium2 Bass kernel for MllamaTextSelfAttention (B=1, S=2048, HID=4096,
32 Q heads / 8 KV heads, HD=128, RoPE, causal mask, GQA).

Sharding: tensor-parallel over heads across 8 NeuronCores. Core c computes
Q heads [4c, 4c+4) and KV head c, plus the matching slice of the output
projection; the 8 partial outputs are summed on the host.

All DRAM I/O is bf16 (host converts); matmuls run bf16 at 1 cyc/row.

Device dataflow (per core):
  phase 1 (per 512-seq stripe, pipelined):
    - stream ht k-tiles, interleaved with the wq/wk/wv weight tiles on the
      first stripe (3 DMA queues round-robin) so PE starts within ~1us
    - qT/kT/vT stationary-weight projections -> [d, s] layouts in PSUM
    - PSUM->SBUF copies on Act (bf16), RoPE per stripe on DVE,
      V transposed per stripe via PE into v2 [s, t*(d|1)] with a ones column
    - wo prefetched during stripe 1
  phase 2+3 (per q stripe qs, heads software-pipelined):
    - S^T[k, q] = kT-tile.T @ qT-stripe (k-partition layout); diagonal tiles
      computed only for the causal column range
    - E = exp(S^T + tri-mask) on Act -> bf16 e tiles
    - O[q, d|den] = e-subtile.T @ (v|1)-tile accumulated over k: the ones
      column of v2 makes column 128 the softmax denominator (no separate
      den matmul)
    - normalize on DVE (per-partition reciprocal bcast), transpose O via PE
      back to [d, q] for the output projection
    - phase 3 for this stripe's 4 s-tiles: Y stripe = O^T-tiles.T @ woT,
      streamed to DRAM as bf16
"""

import math
import os
import sys

for _p in (
    "/opt/trn_rl_repo",
    "/root/.axon_site",
    "/root/.axon_site/_ro/trn_rl_repo",
    "/root/.axon_site/_ro/pypackages",
):
    if os.path.isdir(_p) and _p not in sys.path:
        sys.path.append(_p)

import numpy as np
import ml_dtypes
from contextlib import ExitStack

import concourse.bass as bass
import concourse.tile as tile
from concourse import mybir
from concourse.bass_utils import run_bass_kernel_spmd
from concourse.masks import make_identity

F32 = mybir.dt.float32
BF16 = mybir.dt.bfloat16
ACTF = mybir.ActivationFunctionType
MULT = mybir.AluOpType.mult
BF = ml_dtypes.bfloat16

B, S, HID = 1, 2048, 4096
NH, NKV, HD = 32, 8, 128
NCORES = 8
QH = NH // NCORES          # 4 q heads per core
SS = 512                   # sequence stripe (matmul free dim)
NQS = S // SS              # 4 stripes
NKT = S // 128             # 16 k tiles
KH = HID // 128            # 32 hidden-dim k tiles
VW = 130                   # v2 column stride per k-tile (128 d + ones + pad)
NEG = -1e9


def _split_multi_waits(nc: bass.Bass):
    """Walrus in this container encodes at most ONE sync-wait command per
    instruction. Hoist extra waits onto injected same-engine NoOps placed
    immediately before the instruction; engines are in-order so the
    semantics are unchanged."""
    n = 0
    for fn in nc.m.functions:
        for bb in fn.blocks:
            out = []
            for inst in bb.instructions:
                si = inst.sync_info
                if si is not None and si.on_wait and len(si.on_wait) > 1:
                    waits = list(si.on_wait)
                    for w in waits[:-1]:
                        n += 1
                        nop = mybir.InstNoOp(name=f"I-swait-{n}", ins=[], outs=[])
                        nop.engine = inst.engine
                        nop.sync_info = mybir.SyncInfo(on_wait=[w], on_update=[])
                        out.append(nop)
                    si.on_wait = [waits[-1]]
                out.append(inst)
            bb.instructions[:] = out
    return nc


_BUILD_CACHE = {}


def _build(causal: bool, split_waits: bool = True, loop_n=None) -> bass.Bass:
    key = (causal, split_waits, loop_n)
    if key in _BUILD_CACHE:
        return _BUILD_CACHE[key]

    nc = bass.Bass()
    # all bulk tensors are host-packed into SBUF layout [128, W] so every DMA
    # is a single wide uniform-stride copy (minimal descriptors/instructions)
    hTp = nc.dram_tensor("hTp", [128, NQS * KH * SS], BF16, kind="ExternalInput")
    wqp = nc.dram_tensor("wqp", [128, KH * 512], BF16, kind="ExternalInput")
    wkvp = nc.dram_tensor("wkvp", [128, KH * 256], BF16, kind="ExternalInput")
    wop = nc.dram_tensor("wop", [128, QH * HID], BF16, kind="ExternalInput")
    csp = nc.dram_tensor("csp", [HD, 2 * S], BF16, kind="ExternalInput")
    tri = nc.dram_tensor("tri", [128, 128], F32, kind="ExternalInput")
    if not causal:
        maskT = nc.dram_tensor("maskT", [S, S], F32, kind="ExternalInput")
    y = nc.dram_tensor("y", [S, HID], BF16, kind="ExternalOutput")

    with tile.TileContext(nc) as tc, ExitStack() as ctx:
        if loop_n is not None:
            ctx.enter_context(tc.For_i(0, loop_n, 1))
        # the two HWDGE-capable issue queues, round-robined for bulk DMA
        # (gpsimd/SWDGE is avoided: it can't run inside For_i)
        DQ = [nc.sync, nc.scalar]
        qi = [0]

        def dq():
            e = DQ[qi[0] % len(DQ)]
            qi[0] += 1
            return e

        outer = ctx.enter_context(tc.tile_pool(name="outer", bufs=1))
        ppt = ctx.enter_context(tc.tile_pool(name="ppt", bufs=2, space="PSUM"))

        qT = outer.tile([128, QH * S], BF16)     # [d, h*s]
        kT = outer.tile([128, S], BF16)          # [d, s]
        v2 = outer.tile([128, NKT * VW], BF16)   # [s-in-tile, t*(d|1|pad)]
        ot = outer.tile([128, QH * S], BF16)     # [d, h*s] normalized O^T
        cs_sb = outer.tile([128, 2 * S], BF16)   # cos | sin, packed
        wo_sb = outer.tile([128, QH * HID], BF16)
        id_bf = outer.tile([128, 128], BF16)
        tri_sb = outer.tile([128, 128], F32)
        # stripe-3 V tile + rope temps live in the outer pool: their rope /
        # transpose work is deferred into the attention phase (behind qs=0)
        # so the qs=0 mask-adds aren't queued behind 25 DVE rope ops
        vt3 = outer.tile([128, SS], BF16)
        rt1 = outer.tile([128, SS], BF16)
        rt2 = outer.tile([128, SS], BF16)

        make_identity(nc, id_bf[:])
        for t in range(NKT):
            nc.gpsimd.memset(v2[:, t * VW + 128 : t * VW + 129], 1.0)

        def rope(dst, n, t1, t2):
            """RoPE in [d, s] layout on a [128, SS] bf16 stripe, in place.
            rotate_half: rot[0:64] = -src[64:128]; rot[64:128] = src[0:64].
            dst = dst*cos + rot(dst)*sin"""
            c = cs_sb[:, n * SS : (n + 1) * SS]
            sn = cs_sb[:, S + n * SS : S + (n + 1) * SS]
            nc.vector.tensor_scalar_mul(t1[0:64, :], dst[64:128, :], -1.0)
            nc.vector.tensor_copy(t1[64:128, :], dst[0:64, :])
            nc.vector.tensor_mul(t1[:], t1[:], sn)
            nc.vector.tensor_mul(t2[:], dst, c)
            nc.vector.tensor_add(dst, t1[:], t2[:])

        def v_transpose(vt, n):
            for j in range(4):
                t = 4 * n + j
                pt = ppt.tile([128, 128], BF16, tag="pt")
                nc.tensor.transpose(pt[:], vt[:, j * 128 : (j + 1) * 128], id_bf[:])
                nc.vector.tensor_copy(v2[:, t * VW : t * VW + 128], pt[:])

        # ---------------- phase 1: QKV projections (pipelined) -------------
        with (
            tc.tile_pool(name="wqkv", bufs=1) as wp,
            tc.tile_pool(name="hstream", bufs=3) as hp,
            tc.tile_pool(name="vtmp", bufs=2) as vp,
            tc.tile_pool(name="rtmp", bufs=2) as rp,
            tc.tile_pool(name="ps1", bufs=1, space="PSUM") as pp1,
        ):
            wq_sb = wp.tile([128, KH * 512], BF16)
            wkv_sb = wp.tile([128, KH * 256], BF16)

            vts = []  # pending (vtile, n) for PE transposes

            for n in range(NQS):
                psq = [
                    pp1.tile([128, SS], F32, name=f"psq{m}", tag=f"psq{m}")
                    for m in range(QH)
                ]
                psk = pp1.tile([128, SS], F32, tag="psk")
                psv = pp1.tile([128, SS], F32, tag="psv")
                for kp in range(KH // 2):
                    # pair-granular DMA: weights (stripe 0 only) + h stripe
                    ht2 = hp.tile([128, 2 * SS], BF16, tag="ht2")
                    if n == 0:
                        # fixed queue roles so the first pair lands in ~1.2us
                        nc.sync.dma_start(
                            wq_sb[:, kp * 1024 : (kp + 1) * 1024],
                            wqp[:, kp * 1024 : (kp + 1) * 1024],
                        )
                        nc.scalar.dma_start(
                            wkv_sb[:, kp * 512 : (kp + 1) * 512],
                            wkvp[:, kp * 512 : (kp + 1) * 512],
                        )
                        (nc.scalar if kp % 2 == 0 else nc.sync).dma_start(
                            ht2[:],
                            hTp[:, (n * KH + 2 * kp) * SS : (n * KH + 2 * kp + 2) * SS],
                        )
                    else:
                        dq().dma_start(
                            ht2[:],
                            hTp[:, (n * KH + 2 * kp) * SS : (n * KH + 2 * kp + 2) * SS],
                        )
                    if n == 0 and kp == 14:
                        # RoPE tables + mask: first read by stripe-0 RoPE just
                        # after this stripe; by kp=14 the queues run ~9us
                        # ahead of PE so this doesn't stall the chase
                        dq().dma_start(cs_sb[:], csp[:, :])
                        dq().dma_start(tri_sb[:], tri[:, :])
                    # prefetch wo during stripe 1 (queues are idle by then)
                    if n == 1 and kp % 8 == 4:
                        hh = kp // 8 * 2
                        dq().dma_start(
                            wo_sb[:, hh * HID : (hh + 2) * HID],
                            wop[:, hh * HID : (hh + 2) * HID],
                        )
                    for kk in range(2):
                        k = 2 * kp + kk
                        st, sp = (k == 0), (k == KH - 1)
                        hts = ht2[:, kk * SS : (kk + 1) * SS]
                        for m in range(QH):
                            nc.tensor.matmul(
                                psq[m][:],
                                wq_sb[:, k * 512 + m * 128 : k * 512 + (m + 1) * 128],
                                hts,
                                start=st,
                                stop=sp,
                            )
                        nc.tensor.matmul(
                            psk[:],
                            wkv_sb[:, k * 256 : k * 256 + 128],
                            hts,
                            start=st,
                            stop=sp,
                        )
                        nc.tensor.matmul(
                            psv[:],
                            wkv_sb[:, k * 256 + 128 : (k + 1) * 256],
                            hts,
                            start=st,
                            stop=sp,
                        )
                # PSUM -> SBUF (bf16) on Act, then RoPE stripe on DVE
                for m in range(QH):
                    nc.scalar.copy(
                        qT[:, m * S + n * SS : m * S + (n + 1) * SS], psq[m][:]
                    )
                nc.scalar.copy(kT[:, n * SS : (n + 1) * SS], psk[:])
                vt = vt3 if n == NQS - 1 else vp.tile([128, SS], BF16, tag="vt")
                nc.scalar.copy(vt[:], psv[:])
                if n < NQS - 1:
                    # stripe 3's rope + V transpose are deferred into the
                    # attention phase (emitted behind qs=0)
                    t1 = rp.tile([128, SS], BF16, tag="t1")
                    t2 = rp.tile([128, SS], BF16, tag="t2")
                    for m in range(QH):
                        rope(qT[:, m * S + n * SS : m * S + (n + 1) * SS], n, t1, t2)
                    rope(kT[:, n * SS : (n + 1) * SS], n, t1, t2)
                # delay transposes one stripe so the Act copy has drained
                vts.append((vt, n))
                if len(vts) > 1 and vts[0][1] < NQS - 1:
                    v_transpose(*vts.pop(0))
            while vts and vts[0][1] < NQS - 1:
                v_transpose(*vts.pop(0))

        # ---------------- phase 2+3: attention + output projection ---------
        with (
            tc.tile_pool(name="epool", bufs=2) as ep,
            tc.tile_pool(name="onat", bufs=2) as op_,
            tc.tile_pool(name="recp", bufs=4) as rcp,
            tc.tile_pool(name="mrowp", bufs=1) as mp,
            tc.tile_pool(name="yout", bufs=2) as yp,
            tc.tile_pool(name="ps2s", bufs=2, space="PSUM") as pp2s,
            tc.tile_pool(name="ps2o", bufs=2, space="PSUM") as pp2o,
            tc.tile_pool(name="ps3", bufs=2, space="PSUM") as pp3,
        ):
            for qs in range(NQS):
                nkt = 4 * qs + 4 if causal else NKT
                if not causal:
                    mrow = mp.tile([128, NKT * SS], F32, tag="mrow")
                    for t in range(NKT):
                        dq().dma_start(
                            mrow[:, t * SS : (t + 1) * SS],
                            maskT[t * 128 : (t + 1) * 128, qs * SS : (qs + 1) * SS],
                        )

                etiles = {}

                def emit_scores(h):
                    e = ep.tile([128, NKT * SS], BF16, tag="e")
                    etiles[h] = e
                    qsl = qT[:, h * S + qs * SS : h * S + (qs + 1) * SS]
                    for t in range(nkt):
                        pss = pp2s.tile([128, SS], F32, tag="pss")
                        diag_j = t - 4 * qs if causal and t >= 4 * qs else None
                        if diag_j is not None and diag_j > 0:
                            # columns < 128*diag_j are strictly above the
                            # causal diagonal and never read downstream
                            c0 = 128 * diag_j
                        else:
                            c0 = 0
                        nc.tensor.matmul(
                            pss[:, c0:SS],
                            kT[:, t * 128 : (t + 1) * 128],
                            qsl[:, c0:SS],
                            start=True,
                            stop=True,
                        )
                        dst = e[:, t * SS + c0 : (t + 1) * SS]
                        if diag_j is not None:
                            nc.vector.tensor_add(
                                pss[:, c0 : c0 + 128],
                                pss[:, c0 : c0 + 128],
                                tri_sb[:],
                            )
                            nc.scalar.activation(dst, pss[:, c0:SS], ACTF.Exp)
                        elif not causal:
                            nc.vector.tensor_add(
                                pss[:], pss[:], mrow[:, t * SS : (t + 1) * SS]
                            )
                            nc.scalar.activation(dst, pss[:], ACTF.Exp)
                        else:
                            nc.scalar.activation(dst, pss[:], ACTF.Exp)

                def emit_ph3(j):
                    # output projection for s-tile 4*qs+j (all heads' ot ready)
                    st = 4 * qs + j
                    yt = yp.tile([128, HID], BF16, tag="yt")
                    for nn in range(HID // SS):
                        psy = pp3.tile([128, SS], F32, tag="psy")
                        for hh in range(QH):
                            nc.tensor.matmul(
                                psy[:],
                                ot[:, hh * S + st * 128 : hh * S + (st + 1) * 128],
                                wo_sb[:, hh * HID + nn * SS : hh * HID + (nn + 1) * SS],
                                start=(hh == 0),
                                stop=(hh == QH - 1),
                            )
                        if nn % 2 == 0:
                            nc.scalar.copy(yt[:, nn * SS : (nn + 1) * SS], psy[:])
                        else:
                            nc.vector.tensor_copy(yt[:, nn * SS : (nn + 1) * SS], psy[:])
                    nc.sync.dma_start(y[st * 128 : (st + 1) * 128, :], yt[:])

                def emit_pv(h, tail=False):
                    e = etiles.pop(h)
                    pend = []
                    for j in range(QH):
                        qt = 4 * qs + j          # global q 128-tile index
                        nt = qt + 1 if causal else NKT
                        po = pp2o.tile([128, 132], F32, tag="po")
                        for t in range(nt):
                            nc.tensor.matmul(
                                po[:, 0:129],
                                e[:, t * SS + j * 128 : t * SS + (j + 1) * 128],
                                v2[:, t * VW : t * VW + 129],
                                start=(t == 0),
                                stop=(t == nt - 1),
                            )
                        rec = rcp.tile([128, 1], F32, tag="rec")
                        nc.vector.reciprocal(rec[:], po[:, 128:129])
                        on = op_.tile([128, 128], BF16, tag="on")
                        nc.vector.tensor_scalar_mul(on[:], po[:, 0:128], rec[:, 0:1])
                        pt = ppt.tile([128, 128], BF16, tag="pt")
                        nc.tensor.transpose(pt[:], on[:], id_bf[:])
                        nc.vector.tensor_copy(
                            ot[:, h * S + qt * 128 : h * S + (qt + 1) * 128], pt[:]
                        )
                        if tail:
                            # interleave this stripe's output projection one
                            # subtile behind the last head's PV chains
                            if pend:
                                emit_ph3(pend.pop(0))
                            pend.append(j)
                    for j in pend:
                        emit_ph3(j)

                # software-pipeline heads: scores(h+1) overlaps exp(h) drain
                emit_scores(0)
                for h in range(1, QH):
                    emit_scores(h)
                    emit_pv(h - 1)
                emit_pv(QH - 1, tail=True)

                if qs == 0:
                    # deferred stripe-3 rope + V transpose (DVE/PE have slack
                    # here; results are first read by qs=3)
                    nlast = NQS - 1
                    for m in range(QH):
                        rope(qT[:, m * S + nlast * SS : m * S + (nlast + 1) * SS],
                             nlast, rt1, rt2)
                    rope(kT[:, nlast * SS : (nlast + 1) * SS], nlast, rt1, rt2)
                    v_transpose(vt3, nlast)

    if split_waits:
        _split_multi_waits(nc)
    _BUILD_CACHE[key] = nc
    return nc


def _causal_mask_ref() -> np.ndarray:
    return np.triu(np.full((S, S), NEG, np.float32), k=1)


def _tri_mask() -> np.ndarray:
    p = np.arange(128, dtype=np.int64)[:, None]
    f = np.arange(128, dtype=np.int64)[None, :]
    return np.where(p > f, np.float32(NEG), np.float32(0.0)).astype(np.float32)


def make_in_maps(hidden_states, attention_mask, cos, sin, wq, wk, wv, wo):
    """Host-side sharding/preprocessing. Returns (causal, in_maps)."""
    h = np.ascontiguousarray(np.asarray(hidden_states, dtype=np.float32)[0])
    m2 = np.ascontiguousarray(np.asarray(attention_mask, dtype=np.float32)[0, 0])
    wq = np.asarray(wq, dtype=np.float32)
    wk = np.asarray(wk, dtype=np.float32)
    wv = np.asarray(wv, dtype=np.float32)
    wo = np.asarray(wo, dtype=np.float32)

    causal = bool(np.array_equal(m2, _causal_mask_ref()))

    def pack(xT):
        """[R, C] (R = 128*nk) -> SBUF layout [128, nk*C]:
        out[p, k*C + c] = xT[128*k + p, c]."""
        r, cc = xT.shape
        nk = r // 128
        return np.ascontiguousarray(
            xT.reshape(nk, 128, cc).transpose(1, 0, 2).reshape(128, nk * cc)
        )

    hT = h.T.astype(BF)                                       # [HID, S]
    # hTp[p, (n*KH + k)*SS + c] = h[s=n*SS+c, hid=128k+p]
    hTp = np.ascontiguousarray(
        hT.reshape(KH, 128, NQS, SS).transpose(1, 2, 0, 3).reshape(128, NQS * KH * SS)
    )
    cosT = np.asarray(cos, dtype=np.float32)[0].T.astype(BF)
    sinT = np.asarray(sin, dtype=np.float32)[0].T.astype(BF)
    csp = np.ascontiguousarray(np.concatenate([cosT, sinT], axis=1))
    sc = np.float32(1.0 / math.sqrt(HD))
    trim = _tri_mask()
    if not causal:
        mT = np.ascontiguousarray(m2.T)

    in_maps = []
    for c in range(NCORES):
        wkT = wk[c * HD : (c + 1) * HD].T.astype(BF)          # [HID, 128]
        wvT = wv[c * HD : (c + 1) * HD].T.astype(BF)
        # wkvp[p, k*256 + (0:128|128:256)] = wkT|wvT[128k+p, :]
        wkv = np.concatenate(
            [wkT.reshape(KH, 128, HD), wvT.reshape(KH, 128, HD)], axis=2
        ).transpose(1, 0, 2).reshape(128, KH * 256)
        im = {
            "hTp": hTp,
            "csp": csp,
            "tri": trim,
            "wqp": pack((wq[c * QH * HD : (c + 1) * QH * HD] * sc).T.astype(BF)),
            "wkvp": np.ascontiguousarray(wkv),
            "wop": pack(wo[:, c * QH * HD : (c + 1) * QH * HD].T.astype(BF)),
        }
        if not causal:
            im["maskT"] = mT
        in_maps.append(im)
    return causal, in_maps


def kernel(hidden_states, attention_mask, cos, sin, wq, wk, wv, wo):
    causal, in_maps = make_in_maps(
        hidden_states, attention_mask, cos, sin, wq, wk, wv, wo
    )
    nc = _build(causal)
    res = run_bass_kernel_spmd(nc, in_maps, list(range(NCORES)))
    out = np.zeros((S, HID), np.float64)
    for c in range(NCORES):
        out += np.asarray(res.results[c]["y"]).astype(np.float64)
    return out.reshape(B, S, HID).astype(np.float32)
